# revision 1
# baseline (speedup 1.0000x reference)
"""Trainium2 Bass kernel for a 2-layer GAT (PyG GATConv x2, eval mode).

Strategy (8 NeuronCores, SPMD single program):
  - Host: add self-loops, sort destinations by in-degree, pack into groups of
    128 similar-degree dsts, deal groups round-robin to cores. Per-group edge
    slots [128 dsts x degree] are split into LO/HI halves by source table row
    (dma_gather carries int16 indices, so rows >= 32768 gather from a shifted
    table base). All per-core differences ride in input data.
  - Phase A (replicated): T1 = x @ [W1 | W1@Asrc | W1@Adst] bf16 GEMM ->
    fp32 node table (256B-aligned rows) + separate s_dst array.
  - Phase B (edge pass 1): per dst-group bulk dma_gather of source rows,
    scores (leaky-relu/exp) on DVE/ACT, exp-weighted messages summed over
    neighbors via identity-matmul PSUM accumulation, then batched
    softmax-normalize + bias + ELU.
  - Phase C: layer-2 GEMM on own rows, AllGather the layer-2 table.
  - Phase D: edge pass 2 (heads=1), normalize + bias, write output chunk.
  - Host: gather per-core chunks, undo the permutation.
"""

import numpy as np
import ml_dtypes

P = 128
NCORES = 8
SPLIT = 32768          # dma_gather int16 index range per table base

N_NODES = 50000
F_IN = 256
HID = 32
HEADS = 4
CLS = 40
NEG_SLOPE = 0.2
PAD_SSRC = -1000.0     # pad-row source score: exp(0.2*-1000) == 0 exactly
EPS = 1e-16
TW1 = 256              # layer-1 table row bf16 elems (512B), cols: h(128) s(4) pad
TW2 = 64               # layer-2 table row f32 elems (256B), cols: h(40) s(1) pad


class Plan:
    pass


def _pack16(arr):
    """[128, SD] int -> dma_gather idx layout [128, 8*SD]: index i=(c*128+p)
    at [p%16, c*8 + p//16], and the 16-partition pattern replicated 8x
    across partitions (one copy per Q7 core)."""
    p128, sd = arr.shape
    assert p128 == 128
    base = np.ascontiguousarray(
        arr.reshape(8, 16, sd).transpose(1, 2, 0).reshape(16, sd * 8)
    ).astype(np.int16)
    return np.tile(base, (8, 1))


def make_plan(edge_index, n_nodes, ncores=NCORES, negpad=False):
    import math

    src_all = np.concatenate([edge_index[0], np.arange(n_nodes, dtype=np.int64)])
    dst_all = np.concatenate([edge_index[1], np.arange(n_nodes, dtype=np.int64)])

    deg = np.bincount(dst_all, minlength=n_nodes)
    ngrp = math.ceil(n_nodes / P)
    ngrp = math.ceil(ngrp / ncores) * ncores
    nslot = ngrp * P
    ndum = nslot - n_nodes
    ngc = ngrp // ncores
    nloc = ngc * P
    n_t = nslot + 2                 # rows: [pad_lo, nodes..., pad_hi]
    pad_lo_row = 0
    pad_hi_row = nslot + 1

    order = np.argsort(deg, kind="stable")
    slot_node = np.concatenate([np.full(ndum, -1, np.int64), order])
    groups = slot_node.reshape(ngrp, P)

    # canonical order (core-major) for T2 / output
    can_slot = np.empty((ncores, ngc, P), np.int64)
    for k in range(ncores):
        can_slot[k] = groups[k::ncores]
    can_nodes = can_slot.reshape(-1)
    pos_can = np.full(n_nodes, -1, np.int64)
    m = can_nodes >= 0
    pos_can[can_nodes[m]] = np.nonzero(m)[0]

    # per-core L1 order: own groups first
    order1_nodes, pos1 = [], []
    for k in range(ncores):
        own = groups[k::ncores].reshape(-1)
        rest = np.concatenate([groups[kk::ncores].reshape(-1)
                               for kk in range(ncores) if kk != k])
        o = np.concatenate([own, rest])
        order1_nodes.append(o)
        pm = np.full(n_nodes, -1, np.int64)
        mm = o >= 0
        pm[o[mm]] = np.nonzero(mm)[0]
        pos1.append(pm)

    es = np.argsort(dst_all, kind="stable")
    srcs_sorted = src_all[es]
    starts = np.concatenate([[0], np.cumsum(deg)])

    # per-layer slot structure
    def build_layer(posmaps):
        # posmaps[k]: node -> table pos (0-based, row = pos+1)
        rowsL = [[] for _ in range(ncores)]   # per core: list per gi of [128,w] rows
        loCnt = np.zeros((ncores, ngc, P), np.int64)
        hiCnt = np.zeros((ncores, ngc, P), np.int64)
        blocks = {}
        wmax = int(deg.max())
        cols = np.arange(wmax)
        for k in range(ncores):
            pm = posmaps[k]
            for gi in range(ngc):
                g = gi * ncores + k
                vv = groups[g]
                vv0 = np.maximum(vv, 0)
                L = np.where(vv >= 0, deg[vv0], 0)
                mask = cols[None, :] < L[:, None]
                gpos = starts[vv0][:, None] + cols[None, :]
                ss = srcs_sorted[np.where(mask, gpos, 0)]
                rows = pm[ss] + 1
                lom = mask & (rows < SPLIT)
                him = mask & (rows >= SPLIT)
                loCnt[k, gi] = lom.sum(1)
                hiCnt[k, gi] = him.sum(1)
                ordA = np.argsort(~lom, axis=1, kind="stable")
                pkA = np.take_along_axis(np.where(lom, rows, 0), ordA, 1)
                ordB = np.argsort(~him, axis=1, kind="stable")
                pkB = np.take_along_axis(np.where(him, rows, 0), ordB, 1)
                blocks[(k, gi)] = (pkA, pkB)
        DA = np.maximum(loCnt.max(axis=(0, 2)), 1)
        DB = hiCnt.max(axis=(0, 2))
        offA = np.concatenate([[0], np.cumsum(DA)])
        offB = np.concatenate([[0], np.cumsum(DB)])
        SDA, SDB = int(offA[-1]), int(offB[-1])
        padA = -1 if negpad else pad_lo_row
        padB = -1 if negpad else (pad_hi_row - SPLIT)
        idxlo = np.full((ncores, P, SDA), padA, np.int64)
        idxhi = np.full((ncores, P, SDB), padB, np.int64)
        D = DA + DB
        off1 = np.concatenate([[0], np.cumsum(D)])
        SD1 = int(off1[-1])
        msk = np.zeros((ncores, P, SD1), np.float32)
        for k in range(ncores):
            for gi in range(ngc):
                pkA, pkB = blocks[(k, gi)]
                da, db = int(DA[gi]), int(DB[gi])
                a = np.zeros((P, da), np.int64)
                wa = min(da, pkA.shape[1])
                a[:, :wa] = pkA[:, :wa]
                cm = np.arange(da)[None, :] >= loCnt[k, gi][:, None]
                a[cm] = padA
                idxlo[k, :, offA[gi]:offA[gi] + da] = a
                msk[k, :, off1[gi]:off1[gi] + da] = (~cm).astype(np.float32)
                if db:
                    b = np.full((P, db), padB + (SPLIT if not negpad else 0),
                                np.int64)
                    wb = min(db, pkB.shape[1])
                    b[:, :wb] = pkB[:, :wb]
                    cmb = np.arange(db)[None, :] >= hiCnt[k, gi][:, None]
                    b[cmb] = padB + (SPLIT if not negpad else 0)
                    idxhi[k, :, offB[gi]:offB[gi] + db] = (
                        b - SPLIT if not negpad else np.where(b < 0, b, b - SPLIT))
                    msk[k, :, off1[gi] + da:off1[gi] + da + db] = (
                        (~cmb).astype(np.float32))
        if negpad:
            # trailing -1s in a call are skipped (slot left unwritten) and an
            # all-negative call wedges the Q7 - pin partitions 0 and 127 of
            # every slot column to the inert pad row
            for pp in (0, 16, 32, 48, 64, 80, 96, 112, 127):
                idxlo[:, pp, :] = np.where(idxlo[:, pp, :] == -1, pad_lo_row,
                                           idxlo[:, pp, :])
                if SDB:
                    idxhi[:, pp, :] = np.where(idxhi[:, pp, :] == -1,
                                               pad_hi_row - SPLIT,
                                               idxhi[:, pp, :])
        lo16 = np.stack([_pack16(idxlo[k]) for k in range(ncores)])
        hi16 = (np.stack([_pack16(idxhi[k]) for k in range(ncores)])
                if SDB else np.zeros((ncores, 128, 8), np.int16))
        return dict(DA=DA, DB=DB, offA=offA, offB=offB, SDA=SDA, SDB=SDB,
                    lo16=lo16, hi16=hi16, msk=msk, off1=off1, SD1=SD1)

    L1 = build_layer(pos1)
    L2 = build_layer([pos_can] * ncores)

    pl = Plan()
    pl.ncores = ncores
    pl.n_nodes = n_nodes
    pl.ngrp, pl.nslot, pl.ngc, pl.nloc, pl.n_t = ngrp, nslot, ngc, nloc, n_t
    pl.L1, pl.L2 = L1, L2
    pl.can_slot = can_slot
    pl.order1_nodes = order1_nodes
    return pl


def make_inputs(pl, x, W1, att_src1, att_dst1, b1, W2, att_src2, att_dst2, b2):
    f_in = x.shape[1]
    fh = W1.shape[1]
    heads = att_src1.shape[0]
    hid = fh // heads
    cls = W2.shape[1]
    cw1g = fh + 2 * heads
    cw2g = cls + 2

    asrc = np.zeros((fh, heads), np.float32)
    adst = np.zeros((fh, heads), np.float32)
    for h in range(heads):
        asrc[h * hid:(h + 1) * hid, h] = att_src1[h]
        adst[h * hid:(h + 1) * hid, h] = att_dst1[h]
    W1a = np.concatenate([W1, W1 @ asrc, W1 @ adst], axis=1)
    kt = f_in // P
    W1a = W1a.reshape(kt, P, cw1g).astype(ml_dtypes.bfloat16)

    W2a = np.concatenate(
        [W2, (W2 @ att_src2[0])[:, None], (W2 @ att_dst2[0])[:, None]], axis=1
    ).astype(np.float32)

    pad1 = np.zeros((1, TW1), np.float32)
    pad1[0, fh:fh + heads] = PAD_SSRC
    pad1 = pad1.astype(ml_dtypes.bfloat16)
    pad2 = np.zeros((1, TW2), np.float32)
    pad2[0, cls] = PAD_SSRC

    b1t = np.tile(b1[None, :], (P, 1)).astype(np.float32)
    b2t = np.tile(b2[None, :], (P, 1)).astype(np.float32)
    idbf = np.eye(P).astype(ml_dtypes.bfloat16)
    idf = np.eye(P, dtype=np.float32)

    in_maps = []
    for k in range(pl.ncores):
        o = pl.order1_nodes[k]
        xtab = np.zeros((pl.nslot, f_in), np.float32)
        mm = o >= 0
        xtab[mm] = x[o[mm]]
        xT = np.ascontiguousarray(xtab.T).astype(ml_dtypes.bfloat16)
        in_maps.append({
            "xT": xT, "W1a": W1a, "W2a": W2a,
            "i1lo": pl.L1["lo16"][k], "i1hi": pl.L1["hi16"][k],
            "i2lo": pl.L2["lo16"][k], "i2hi": pl.L2["hi16"][k],
            "msk1": pl.L1["msk"][k], "msk2": pl.L2["msk"][k],
            "pad1": pad1, "pad2": pad2,
            "b1t": b1t, "b2t": b2t, "idbf": idbf, "idf": idf,
        })
    return in_maps


# ------------------------------------------------------------- bass program

def build_bass(pl, f_in=F_IN, heads=HEADS, hid=HID, cls=CLS, dbg=False, stop_after=None, reps=1):
    import concourse.bass as bass
    import concourse.bacc as bacc
    import concourse.tile as tile
    from concourse import mybir

    fh = heads * hid
    cw1g = fh + 2 * heads
    cw2 = cls + 1
    cw2g = cls + 2
    kt = f_in // P
    ngc, nslot, nloc, n_t = pl.ngc, pl.nslot, pl.nloc, pl.n_t
    ngrp = pl.ngrp
    L1, L2 = pl.L1, pl.L2
    core_ids = list(range(pl.ncores))

    ablk = 1
    for cand in range(min(28, ngrp), 0, -1):
        if ngrp % cand == 0:
            ablk = cand
            break
    nblk = ngrp // ablk

    f32, bf16, i16 = mybir.dt.float32, mybir.dt.bfloat16, mybir.dt.int16
    AF = mybir.ActivationFunctionType
    OP = mybir.AluOpType

    nc = bacc.Bacc("TRN2", target_bir_lowering=False, debug=False)

    xT = nc.declare_dram_parameter("xT", [f_in, nslot], bf16, isOutput=False)
    W1a = nc.declare_dram_parameter("W1a", [kt, P, cw1g], bf16, isOutput=False)
    W2a = nc.declare_dram_parameter("W2a", [fh, cw2g], f32, isOutput=False)
    i1lo = nc.declare_dram_parameter("i1lo", list(L1["lo16"].shape[1:]), i16, isOutput=False)
    i1hi = nc.declare_dram_parameter("i1hi", list(L1["hi16"].shape[1:]), i16, isOutput=False)
    i2lo = nc.declare_dram_parameter("i2lo", list(L2["lo16"].shape[1:]), i16, isOutput=False)
    i2hi = nc.declare_dram_parameter("i2hi", list(L2["hi16"].shape[1:]), i16, isOutput=False)
    msk1 = nc.declare_dram_parameter("msk1", [P, L1["SD1"]], f32, isOutput=False)
    msk2 = nc.declare_dram_parameter("msk2", [P, L2["SD1"]], f32, isOutput=False)
    pad1 = nc.declare_dram_parameter("pad1", [1, TW1], bf16, isOutput=False)
    pad2 = nc.declare_dram_parameter("pad2", [1, TW2], f32, isOutput=False)
    b1t = nc.declare_dram_parameter("b1t", [P, fh], f32, isOutput=False)
    b2t = nc.declare_dram_parameter("b2t", [P, cls], f32, isOutput=False)
    idbf = nc.declare_dram_parameter("idbf", [P, P], bf16, isOutput=False)
    idf = nc.declare_dram_parameter("idf", [P, P], f32, isOutput=False)

    out2d = nc.declare_dram_parameter("out2d", [nloc, cls], f32, isOutput=True)
    if dbg:
        t1o = nc.declare_dram_parameter("t1o", [n_t, TW1], bf16, isOutput=True)
        sd1o = nc.declare_dram_parameter("sd1o", [nslot, heads], bf16, isOutput=True)
        h1o = nc.declare_dram_parameter("h1o", [P, ngc * fh], f32, isOutput=True)
        dso = nc.declare_dram_parameter("dso", [P, ngc * heads], f32, isOutput=True)
        t2o = nc.declare_dram_parameter("t2o", [n_t, TW2], f32, isOutput=True)

    T1 = nc.dram_tensor("T1", [n_t, TW1], bf16)
    SD1 = nc.dram_tensor("SD1", [nslot, heads], bf16)
    T2chunk = nc.dram_tensor("T2chunk", [nloc, TW2], f32)
    T2 = nc.dram_tensor("T2", [n_t, TW2], f32, addr_space="Shared")

    def ap_of(t, offset, dims):
        a = t[:]
        part = list(a.ap[0])
        return bass.AP(a.tensor, a.offset + offset, [part] + [list(d) for d in dims])

    def mkap(t, offset, dims):
        a = t[:]
        return bass.AP(a.tensor, a.offset + offset, [list(d) for d in dims])

    MAXSLOT = 8        # dma_gather caps at 1024 indices per call

    def gather(out_tile, slot_off, nslots, table, tw, idx_tile, idx_off, base_row):
        """dma_gather nslots*128 rows of width tw into out_tile at slot_off."""
        in_ap = mkap(table, base_row * tw, [[tw, n_t - base_row], [1, tw]])
        done = 0
        while done < nslots:
            cn = min(MAXSLOT, nslots - done)
            out_ap = ap_of(out_tile, (slot_off + done) * tw, [[tw, cn], [1, tw]])
            idx_ap = idx_tile[:, (idx_off + done) * 8:(idx_off + done + cn) * 8]
            n = cn * P
            nc.gpsimd.dma_gather(
                out_ap=out_ap, in_ap=in_ap, idxs_ap=idx_ap,
                num_idxs=n, num_idxs_reg=n, elem_size=tw,
            )
            done += cn

    with tile.TileContext(nc) as tc:
        with tc.tile_pool(name="stage", bufs=1) as stage:
            idbf_t = stage.tile([P, P], bf16)
            nc.sync.dma_start(out=idbf_t[:], in_=idbf[:, :])
            idf_t = stage.tile([P, P], f32)
            nc.sync.dma_start(out=idf_t[:], in_=idf[:, :])
            b1t_t = stage.tile([P, fh], f32)
            nc.sync.dma_start(out=b1t_t[:], in_=b1t[:, :])
            b2t_t = stage.tile([P, cls], f32)
            nc.sync.dma_start(out=b2t_t[:], in_=b2t[:, :])

            numstage = stage.tile([P, ngc * fh], f32)
            tmpstage = stage.tile([P, ngc * fh], f32)
            dstage = stage.tile([P, ngc * heads], f32)
            s2d = stage.tile([P, ngc], f32)
            d2stage = stage.tile([P, ngc], f32)
            o2stage = stage.tile([P, ngc * cls], f32)

            for _rep in range(reps):
                # ------------------------------------------------ phase A: T1 GEMM
                with (
                    tc.tile_pool(name="pa", bufs=2) as pa,
                    tc.tile_pool(name="pa_w", bufs=1) as paw,
                    tc.tile_pool(name="psA", bufs=4, space="PSUM") as psA,
                ):
                    w1_sb = paw.tile([P, kt * cw1g], bf16)
                    nc.sync.dma_start(
                        out=w1_sb[:], in_=W1a[:, :, :].transpose([1, 0, 2])
                    )
                    nc.sync.dma_start(out=T1[0:1, :], in_=pad1[:, :])
                    nc.sync.dma_start(out=T1[nslot + 1:nslot + 2, :], in_=pad1[:, :])

                    for blk in range(nblk):
                        c0 = blk * ablk * P
                        xa = []
                        for kk in range(kt):
                            t = pa.tile([P, ablk * P], bf16, tag=f"xa{kk}")
                            nc.sync.dma_start(
                                out=t[:],
                                in_=xT[kk * P:(kk + 1) * P, c0:c0 + ablk * P],
                            )
                            xa.append(t)
                        stb = pa.tile([P, ablk * cw1g], bf16, tag="ast", bufs=2)
                        for m0 in range(0, ablk, 3):
                            nm = min(3, ablk - m0)
                            pt = psA.tile([P, nm * cw1g], f32)
                            for i in range(nm):
                                for kk in range(kt):
                                    nc.tensor.matmul(
                                        out=pt[:, i * cw1g:(i + 1) * cw1g],
                                        lhsT=xa[kk][:, (m0 + i) * P:(m0 + i + 1) * P],
                                        rhs=w1_sb[:, kk * cw1g:(kk + 1) * cw1g],
                                        start=(kk == 0),
                                        stop=(kk == kt - 1),
                                    )
                            nc.scalar.activation(
                                out=stb[:, m0 * cw1g:(m0 + nm) * cw1g],
                                in_=pt[:], func=AF.Copy,
                            )
                        # one T1 DMA + one SD1 DMA per block
                        nc.sync.dma_start(
                            out=mkap(T1, (c0 + 1) * TW1,
                                     [[TW1, P], [P * TW1, ablk], [1, fh + heads]]),
                            in_=ap_of(stb, 0, [[cw1g, ablk], [1, fh + heads]]),
                        )
                        nc.sync.dma_start(
                            out=mkap(SD1, c0 * heads,
                                     [[heads, P], [P * heads, ablk], [1, heads]]),
                            in_=ap_of(stb, fh + heads, [[cw1g, ablk], [1, heads]]),
                        )

                # ---------------------------------------------- phase B: edge pass 1
                with (
                    tc.tile_pool(name="pb", bufs=2) as pb,
                    tc.tile_pool(name="pb_c", bufs=1) as pbc,
                    tc.tile_pool(name="psB", bufs=2, space="PSUM") as psB,
                    tc.tile_pool(name="psT", bufs=2, space="PSUM") as psT,
                ):
                    i1lo_t = pbc.tile([P, L1["lo16"].shape[2]], i16)
                    nc.sync.dma_start(out=i1lo_t[:], in_=i1lo[:, :])
                    i1hi_t = pbc.tile([P, L1["hi16"].shape[2]], i16)
                    nc.sync.dma_start(out=i1hi_t[:], in_=i1hi[:, :])
                    msk1_t = pbc.tile([P, L1["SD1"]], f32)
                    nc.sync.dma_start(out=msk1_t[:], in_=msk1[:, :])
                    sdst_sb = pbc.tile([P, ngc * heads], bf16)
                    nc.sync.dma_start(
                        out=sdst_sb[:],
                        in_=mkap(SD1, 0, [[heads, P], [heads * P, ngc], [1, heads]]),
                    )

                    for gi in range(ngc):
                        da, db = int(L1["DA"][gi]), int(L1["DB"][gi])
                        d = da + db
                        m_t = pb.tile([P, d * TW1], bf16, tag="m")
                        gather(m_t, 0, da, T1, TW1, i1lo_t, int(L1["offA"][gi]), 0)
                        if db:
                            gather(m_t, da, db, T1, TW1, i1hi_t,
                                   int(L1["offB"][gi]), SPLIT)
                        ssum = pb.tile([P, d * heads], f32, tag="ss")
                        nc.vector.tensor_tensor(
                            out=ssum[:],
                            in0=ap_of(m_t, fh, [[TW1, d], [1, heads]]),
                            in1=ap_of(sdst_sb, gi * heads, [[0, d], [1, heads]]),
                            op=OP.add,
                        )
                        tmp = pb.tile([P, d * heads], f32, tag="tm")
                        nc.vector.tensor_scalar_mul(
                            out=tmp[:], in0=ssum[:], scalar1=NEG_SLOPE
                        )
                        nc.vector.tensor_tensor(
                            out=ssum[:], in0=ssum[:], in1=tmp[:], op=OP.max
                        )
                        ex = pb.tile([P, d * heads], f32, tag="ex")
                        nc.scalar.activation(out=ex[:], in_=ssum[:], func=AF.Exp)
                        nc.vector.tensor_tensor(
                            out=ex[:],
                            in0=ex[:],
                            in1=ap_of(msk1_t, int(L1["off1"][gi]),
                                      [[1, d], [0, heads]]),
                            op=OP.mult,
                        )
                        nc.vector.tensor_reduce(
                            out=dstage[:, gi * heads:(gi + 1) * heads],
                            in_=ap_of(ex, 0, [[1, heads], [heads, d]]),
                            axis=mybir.AxisListType.X,
                            op=OP.add,
                        )
                        mw = pb.tile([P, d * fh], bf16, tag="mw")
                        nc.vector.tensor_tensor(
                            out=mw[:],
                            in0=ap_of(m_t, 0, [[TW1, d], [hid, heads], [1, hid]]),
                            in1=ap_of(ex, 0, [[heads, d], [1, heads], [0, hid]]),
                            op=OP.mult,
                        )
                        pn = psB.tile([P, fh], f32)
                        for j in range(d):
                            nc.tensor.matmul(
                                out=pn[:],
                                lhsT=idbf_t[:],
                                rhs=mw[:, j * fh:(j + 1) * fh],
                                start=(j == 0),
                                stop=(j == d - 1),
                            )
                        nc.scalar.activation(
                            out=numstage[:, gi * fh:(gi + 1) * fh],
                            in_=pn[:], func=AF.Copy,
                        )

                    # batched tail: out1 = elu(num/den + b1)
                    nc.vector.tensor_scalar_add(
                        out=dstage[:], in0=dstage[:], scalar1=EPS
                    )
                    nc.vector.reciprocal(out=dstage[:], in_=dstage[:])
                    nc.vector.tensor_tensor(
                        out=numstage[:],
                        in0=ap_of(numstage, 0, [[fh, ngc], [hid, heads], [1, hid]]),
                        in1=ap_of(dstage, 0, [[heads, ngc], [1, heads], [0, hid]]),
                        op=OP.mult,
                    )
                    nc.vector.tensor_tensor(
                        out=numstage[:],
                        in0=numstage[:],
                        in1=ap_of(b1t_t, 0, [[0, ngc], [1, fh]]),
                        op=OP.add,
                    )
                    nc.vector.tensor_scalar_min(
                        out=tmpstage[:], in0=numstage[:], scalar1=0.0
                    )
                    nc.scalar.activation(out=tmpstage[:], in_=tmpstage[:], func=AF.Exp)
                    nc.scalar.activation(
                        out=tmpstage[:], in_=tmpstage[:], func=AF.Copy, bias=-1.0
                    )
                    nc.vector.tensor_tensor(
                        out=numstage[:], in0=numstage[:], in1=tmpstage[:], op=OP.max
                    )
                    if dbg:
                        nc.sync.dma_start(out=t1o[:, :], in_=T1[:, :])
                        nc.sync.dma_start(out=sd1o[:, :], in_=SD1[:, :])
                        nc.sync.dma_start(out=h1o[:, :], in_=numstage[:])
                        nc.sync.dma_start(out=dso[:, :], in_=dstage[:])
                    # in-place per-group transpose: numstage becomes [feat, row]
                    for gi in range(ngc):
                        ptr = psT.tile([P, fh], f32)
                        nc.tensor.transpose(
                            out=ptr[:],
                            in_=numstage[:, gi * fh:(gi + 1) * fh],
                            identity=idf_t[:],
                        )
                        nc.scalar.activation(
                            out=numstage[:, gi * fh:(gi + 1) * fh],
                            in_=ptr[:], func=AF.Copy,
                        )

                # ------------------------------------- phase C: layer-2 GEMM + AllGather
                if stop_after in ("B",):
                    skipC = True
                else:
                    with (
                        tc.tile_pool(name="pc_w", bufs=1) as pcw,
                        tc.tile_pool(name="psC", bufs=4, space="PSUM") as psC,
                    ):
                        w2_sb = pcw.tile([P, cw2g], f32)
                        nc.sync.dma_start(out=w2_sb[:], in_=W2a[:, :])
                        for gi in range(ngc):
                            pc = psC.tile([P, cw2g], f32)
                            nc.tensor.matmul(
                                out=pc[:],
                                lhsT=numstage[:, gi * fh:(gi + 1) * fh],
                                rhs=w2_sb[:],
                                start=True,
                                stop=True,
                            )
                            stc = pcw.tile([P, cw2g], f32, tag="cst", bufs=3)
                            nc.scalar.activation(out=stc[:], in_=pc[:], func=AF.Copy)
                            nc.sync.dma_start(
                                out=mkap(T2chunk, gi * P * TW2, [[TW2, P], [1, cw2]]),
                                in_=stc[:, 0:cw2],
                            )
                            nc.scalar.activation(
                                out=s2d[:, gi:gi + 1], in_=stc[:, cw2:cw2 + 1], func=AF.Copy
                            )
                        nc.gpsimd.collective_compute(
                            "AllGather",
                            OP.bypass,
                            replica_groups=[core_ids],
                            ins=[T2chunk[:, :]],
                            outs=[T2[1:1 + nslot, :]],
                        )
                        nc.sync.dma_start(out=T2[0:1, :], in_=pad2[:, :])
                        nc.sync.dma_start(out=T2[nslot + 1:nslot + 2, :], in_=pad2[:, :])
                        if dbg:
                            nc.sync.dma_start(out=t2o[:, :], in_=T2[:, :])

                # ---------------------------------------------- phase D: edge pass 2
                if stop_after in ("B", "C"):
                    skipD = True
                else:
                    with (
                        tc.tile_pool(name="pd", bufs=2) as pd,
                        tc.tile_pool(name="pd_c", bufs=1) as pdc,
                        tc.tile_pool(name="psD", bufs=2, space="PSUM") as psD,
                    ):
                        i2lo_t = pdc.tile([P, L2["lo16"].shape[2]], i16)
                        nc.sync.dma_start(out=i2lo_t[:], in_=i2lo[:, :])
                        i2hi_t = pdc.tile([P, L2["hi16"].shape[2]], i16)
                        nc.sync.dma_start(out=i2hi_t[:], in_=i2hi[:, :])

                        for gi in range(ngc):
                            da, db = int(L2["DA"][gi]), int(L2["DB"][gi])
                            d = da + db
                            m2 = pd.tile([P, d * TW2], f32, tag="m2")
                            gather(m2, 0, da, T2, TW2, i2lo_t, int(L2["offA"][gi]), 0)
                            if db:
                                gather(m2, da, db, T2, TW2, i2hi_t,
                                       int(L2["offB"][gi]), SPLIT)
                            ssum = pd.tile([P, d], f32, tag="ss2")
                            nc.vector.tensor_tensor(
                                out=ssum[:],
                                in0=ap_of(m2, cls, [[TW2, d]]),
                                in1=ap_of(s2d, gi, [[0, d]]),
                                op=OP.add,
                            )
                            tmp = pd.tile([P, d], f32, tag="tm2")
                            nc.vector.tensor_scalar_mul(
                                out=tmp[:], in0=ssum[:], scalar1=NEG_SLOPE
                            )
                            nc.vector.tensor_tensor(
                                out=ssum[:], in0=ssum[:], in1=tmp[:], op=OP.max
                            )
                            ex = pd.tile([P, d], f32, tag="ex2")
                            nc.scalar.activation(out=ex[:], in_=ssum[:], func=AF.Exp)
                            nc.vector.tensor_reduce(
                                out=d2stage[:, gi:gi + 1],
                                in_=ex[:],
                                axis=mybir.AxisListType.X,
                                op=OP.add,
                            )
                            mw = pd.tile([P, d * cls], bf16, tag="mw2")
                            nc.vector.tensor_tensor(
                                out=mw[:],
                                in0=ap_of(m2, 0, [[TW2, d], [1, cls]]),
                                in1=ap_of(ex, 0, [[1, d], [0, cls]]),
                                op=OP.mult,
                            )
                            pn = psD.tile([P, cls], f32)
                            for j in range(d):
                                nc.tensor.matmul(
                                    out=pn[:],
                                    lhsT=idbf_t[:],
                                    rhs=mw[:, j * cls:(j + 1) * cls],
                                    start=(j == 0),
                                    stop=(j == d - 1),
                                )
                            nc.scalar.activation(
                                out=o2stage[:, gi * cls:(gi + 1) * cls],
                                in_=pn[:], func=AF.Copy,
                            )

                        nc.vector.tensor_scalar_add(
                            out=d2stage[:], in0=d2stage[:], scalar1=EPS
                        )
                        nc.vector.reciprocal(out=d2stage[:], in_=d2stage[:])
                        nc.vector.tensor_tensor(
                            out=o2stage[:],
                            in0=ap_of(o2stage, 0, [[cls, ngc], [1, cls]]),
                            in1=ap_of(d2stage, 0, [[1, ngc], [0, cls]]),
                            op=OP.mult,
                        )
                        nc.vector.tensor_tensor(
                            out=o2stage[:],
                            in0=o2stage[:],
                            in1=ap_of(b2t_t, 0, [[0, ngc], [1, cls]]),
                            op=OP.add,
                        )
                        nc.sync.dma_start(
                            out=mkap(out2d, 0, [[cls, P], [P * cls, ngc], [1, cls]]),
                            in_=ap_of(o2stage, 0, [[cls, ngc], [1, cls]]),
                        )

    nc.compile()
    return nc


def assemble_output(pl, results, n_nodes, cls=CLS):
    out = np.zeros((n_nodes, cls), np.float32)
    for k in range(pl.ncores):
        chunk = results[k]["out2d"]
        nodes = pl.can_slot[k].reshape(-1)
        m = nodes >= 0
        out[nodes[m]] = chunk[m]
    return out


# ----------------------------------------------------------------- entry

def kernel(edge_index, x, W1, att_src1, att_dst1, b1, W2, att_src2, att_dst2, b2):
    x = np.asarray(x, np.float32)
    edge_index = np.asarray(edge_index)
    n_nodes = x.shape[0]

    pl = make_plan(edge_index, n_nodes)
    in_maps = make_inputs(pl, x, np.asarray(W1, np.float32),
                          np.asarray(att_src1, np.float32),
                          np.asarray(att_dst1, np.float32),
                          np.asarray(b1, np.float32),
                          np.asarray(W2, np.float32),
                          np.asarray(att_src2, np.float32),
                          np.asarray(att_dst2, np.float32),
                          np.asarray(b2, np.float32))
    nc = build_bass(pl, f_in=x.shape[1], heads=np.asarray(att_src1).shape[0],
                    hid=np.asarray(att_src1).shape[1],
                    cls=np.asarray(W2).shape[1])

    from concourse.bass_utils import run_bass_kernel_spmd
    res = run_bass_kernel_spmd(nc, in_maps, list(range(NCORES)))
    return assemble_output(pl, res.results, n_nodes,
                           cls=np.asarray(W2).shape[1])

